# revision 9
# baseline (speedup 1.0000x reference)
"""Trainium2 Bass kernel for nn_Cross_transformer (cross-feature sparse attention).

Sharding: 8 cores = (batch b in {0,1}) x (attention pair in {0..3}).
Each core computes, for its (b, pair):
  - the global-branch attention unit  (N=4096 tokens)
  - pair's slice of all 9 patch attention units (N=1024 tokens each)
and the fuse-layer partial y = fuse_block_scaled @ attn_out, which is
AllReduce-summed over the 4 cores of the batch quad. Every core of a quad
then redundantly applies BN/relu/gamma, the overlapping patch scatter-adds,
and the final 3x3 conv; the host takes core 0 (b=0) and core 4 (b=1).

Key algebra used on device (per attention unit, natural [n, m] layout):
  e[n,m] = (Wq Xq)^T (Wkv Xkv);  p = exp(-e) (== softmax(max-e) numerator,
  shift-invariant, range-checked: max(-e) < 70 < fp32 exp limit)
  d[n] = sum_m p[n,m]  (free via ScalarE activation accum_out)
  y_partial[o,m] = sum_n (Xkv^T Wcomb^T)[n,o] / d[n] * p[n,m]
with Wcomb = (diag(bn_scale) @ fuse_w_block) @ Wkv folded on host, so the
fuse matmul and the softmax normalization both fold into the PV operand.
"""

import sys

sys.path.insert(0, "/opt/trn_rl_repo")

from contextlib import ExitStack

import numpy as np

import concourse.bacc as bacc
import concourse.bass as bass
import concourse.mybir as mybir
import concourse.tile as tile
from concourse.bass_utils import run_bass_kernel_spmd

F32 = mybir.dt.float32
AF = mybir.ActivationFunctionType

B, C, H, W = 2, 48, 64, 64
N = H * W            # 4096
h2 = w2 = 32
NP = h2 * w2         # 1024
EPS = 1e-5
ROW = [0, 0, 0, 16, 16, 16, 32, 32, 32]
COL = [0, 16, 32] * 3
NCORES = 8
REPLICA_GROUPS = [[0, 1, 2, 3], [4, 5, 6, 7]]

N_BANDS_G = N // 128     # 32
N_BANDS_P = NP // 128    # 8
MB = 512                 # matmul moving free dim / PSUM bank
PACK_LO = slice(0, 48)   # PV acc partition slot A
PACK_HI = slice(64, 112) # PV acc partition slot B


def _attn_unit(nc, tc, pools, fa_sb, g_sb, wvt_sb, acc_slots, n_bands, m_tot,
               tag):
    """Emit one attention unit's band loop.

    fa_sb  [48, n_tot] query projection (lhsT slices per band)
    g_sb   [48, m_tot] key projection (rhs slices)
    wvt_sb [128, n_bands, 48] folded (Wcomb @ Xkv)^T chunks
    acc_slots: list of (psum_tile, part_slice) of length m_tot//512, the
      persistent PV accumulators for this unit.
    """
    psum_s, dstat = pools["psum_s"], pools["dstat"]
    pband = pools["pband"]
    n_groups = m_tot // 1024
    for band in range(n_bands):
        p_band = pband.tile([128, m_tot], F32, tag="pband")
        d_part = dstat.tile([128, max(n_groups, 1)], F32, tag="d_part")
        lhsT = fa_sb[:, band * 128:(band + 1) * 128]
        for grp in range(n_groups):
            s_t = psum_s.tile([128, 1024], F32, tag="s_t")
            for j in range(2):
                m0 = grp * 1024 + j * MB
                nc.tensor.matmul(
                    s_t[:, j * MB:(j + 1) * MB], lhsT, g_sb[:, m0:m0 + MB],
                    start=True, stop=True,
                )
            nc.scalar.activation(
                out=p_band[:, grp * 1024:(grp + 1) * 1024], in_=s_t[:],
                func=AF.Exp, scale=-1.0,
                accum_out=d_part[:, grp:grp + 1],
            )
        dr = dstat.tile([128, 1], F32, tag="dr")
        if n_groups > 1:
            dsum = dstat.tile([128, 1], F32, tag="dsum")
            nc.vector.reduce_sum(dsum, d_part, axis=mybir.AxisListType.X)
            nc.vector.reciprocal(dr, dsum)
        else:
            nc.vector.reciprocal(dr, d_part)
        wvts = dstat.tile([128, 48], F32, tag="wvts")
        nc.vector.tensor_scalar_mul(wvts, wvt_sb[:, band, :], dr)
        for mb in range(m_tot // MB):
            acc_t, psl = acc_slots[mb]
            nc.tensor.matmul(
                acc_t[psl, :], wvts, p_band[:, mb * MB:(mb + 1) * MB],
                start=(band == 0), stop=(band == n_bands - 1),
                skip_group_check=True,
            )


def _proj(nc, pools, w_lhsT, rhs_slices, out_sb):
    """out_sb[48, n_tot] = W @ X via PE + DVE psum->sbuf copies.

    rhs_slices: list of APs, each with 512 free elements (may be strided)."""
    psum_s = pools["psum_s"]
    for j, rhs in enumerate(rhs_slices):
        ps = psum_s.tile([48, MB], F32, tag="s_t")
        nc.tensor.matmul(ps, w_lhsT, rhs, start=True, stop=True)
        nc.vector.tensor_copy(out_sb[:, j * MB:(j + 1) * MB], ps)


def _wvt(nc, pools, x_sb, wcomb_rhs, wvt_sb, n_bands):
    """wvt_sb[128, n_bands, 48] = (X^T @ Wcomb^T) chunks."""
    psum_s = pools["psum_s"]
    for ch in range(n_bands):
        ps = psum_s.tile([128, 48], F32, tag="s_t")
        nc.tensor.matmul(ps, x_sb[:, ch * 128:(ch + 1) * 128], wcomb_rhs,
                         start=True, stop=True)
        nc.vector.tensor_copy(wvt_sb[:, ch, :], ps)


def build_program():
    nc = bacc.Bacc("TRN2", target_bir_lowering=False, debug=False,
                   enable_asserts=False, num_devices=NCORES)

    def din(name, shape):
        return nc.dram_tensor(name, shape, F32, kind="ExternalInput").ap()

    xq_d = din("xq", [C, N])
    xkv_d = din("xkv", [C, N])
    wq_d = din("wq_lhsT", [C, C])
    wkv_d = din("wkv_lhsT", [C, C])
    wcomb_d = din("wcomb_rhs", [C, C])
    pwq_d = din("pwq_lhsT", [9, C, C])
    pwkv_d = din("pwkv_lhsT", [9, C, C])
    pwcomb_d = din("pwcomb_rhs", [9, C, C])
    fuse_shift_d = din("fuse_shift", [C, 1])
    gvec_all_d = din("gvec_all", [C, 1])
    shiftp_d = din("shiftP", [C, 9])
    gvecp_d = din("gvecP", [C, 9])
    convw_d = din("convw_lhsT", [9, C, C])
    conv_shift_d = din("conv_shift", [C, 1])
    out_d = nc.dram_tensor("out", [C, H, W], F32, kind="ExternalOutput").ap()

    with tile.TileContext(nc) as tc, ExitStack() as ctx:
        pools = {}
        pools["const"] = const = ctx.enter_context(
            tc.tile_pool(name="const", bufs=1))
        pools["feat"] = feat = ctx.enter_context(
            tc.tile_pool(name="feat", bufs=1))
        pools["proj"] = proj = ctx.enter_context(
            tc.tile_pool(name="proj", bufs=1))
        pools["pproj"] = pproj = ctx.enter_context(
            tc.tile_pool(name="pproj", bufs=2))
        pools["crops"] = crops = ctx.enter_context(
            tc.tile_pool(name="crops", bufs=2))
        pools["wvt"] = wvt = ctx.enter_context(
            tc.tile_pool(name="wvt", bufs=1))
        pools["pwvt"] = pwvt = ctx.enter_context(
            tc.tile_pool(name="pwvt", bufs=2))
        pools["pband"] = ctx.enter_context(
            tc.tile_pool(name="pband", bufs=2))
        pools["dstat"] = ctx.enter_context(
            tc.tile_pool(name="dstat", bufs=4))
        pools["ystage"] = ystage = ctx.enter_context(
            tc.tile_pool(name="ystage", bufs=2))
        pools["comb"] = comb = ctx.enter_context(
            tc.tile_pool(name="comb", bufs=2))
        pools["psum_s"] = ctx.enter_context(
            tc.tile_pool(name="psum_s", bufs=2, space="PSUM"))
        pools["psum_acc"] = psum_acc = ctx.enter_context(
            tc.tile_pool(name="psum_acc", bufs=1, space="PSUM"))
        dram = ctx.enter_context(tc.tile_pool(name="dram", bufs=1,
                                              space="DRAM"))

        # ---- constant loads
        wq_sb = const.tile([C, C], F32)
        wkv_sb = const.tile([C, C], F32)
        wcomb_sb = const.tile([C, C], F32)
        pwq_sb = const.tile([C, 9, C], F32)
        pwkv_sb = const.tile([C, 9, C], F32)
        pwcomb_sb = const.tile([C, 9, C], F32)
        fuse_shift_sb = const.tile([C, 1], F32)
        gvec_all_sb = const.tile([C, 1], F32)
        shiftp_sb = const.tile([C, 9], F32)
        gvecp_sb = const.tile([C, 9], F32)
        convw_sb = const.tile([C, 9, C], F32)
        conv_shift_sb = const.tile([C, 1], F32)
        nc.sync.dma_start(out=wq_sb, in_=wq_d)
        nc.sync.dma_start(out=wkv_sb, in_=wkv_d)
        nc.sync.dma_start(out=wcomb_sb, in_=wcomb_d)
        # [9, C, C] -> sbuf [C, 9, C]
        nc.sync.dma_start(out=pwq_sb, in_=pwq_d.rearrange("p a b -> a p b"))
        nc.sync.dma_start(out=pwkv_sb, in_=pwkv_d.rearrange("p a b -> a p b"))
        nc.sync.dma_start(out=pwcomb_sb,
                          in_=pwcomb_d.rearrange("p a b -> a p b"))
        nc.sync.dma_start(out=fuse_shift_sb, in_=fuse_shift_d)
        nc.sync.dma_start(out=gvec_all_sb, in_=gvec_all_d)
        nc.sync.dma_start(out=shiftp_sb, in_=shiftp_d)
        nc.sync.dma_start(out=gvecp_sb, in_=gvecp_d)
        nc.sync.dma_start(out=convw_sb, in_=convw_d.rearrange("p a b -> a p b"))
        nc.sync.dma_start(out=conv_shift_sb, in_=conv_shift_d)

        xq_sb = feat.tile([C, N], F32)
        xkv_sb = feat.tile([C, N], F32)
        nc.sync.dma_start(out=xq_sb, in_=xq_d)
        nc.sync.dma_start(out=xkv_sb, in_=xkv_d)
        xq_hw = xq_sb.rearrange("c (h w) -> c h w", h=H)
        xkv_hw = xkv_sb.rearrange("c (h w) -> c h w", h=H)

        # ---- canvas for the combine, interior filled later
        canvas = proj.tile([C, H + 2, W + 2], F32, tag="canvas")
        nc.vector.memset(canvas, 0.0)

        ar_in_p = dram.tile([C, 9 * NP], F32)
        ar_out_p = dram.tile([C, 9 * NP], F32)
        ar_in_g = dram.tile([C, N], F32)
        ar_out_g = dram.tile([C, N], F32)

        # ================= patch units =================
        for p in range(9):
            r, c0 = ROW[p], COL[p]
            xkv_c = crops.tile([C, NP], F32, tag="xkv_c")
            nc.gpsimd.tensor_copy(
                out=xkv_c.rearrange("c (a b) -> c a b", a=h2),
                in_=xkv_hw[:, r:r + h2, c0:c0 + w2])
            fa_p = pproj.tile([C, NP], F32, tag="fa_p")
            g_p = pproj.tile([C, NP], F32, tag="g_p")
            q_slices = [xq_hw[:, r + 16 * j:r + 16 * (j + 1), c0:c0 + w2]
                        for j in range(2)]
            _proj(nc, pools, pwq_sb[:, p, :], q_slices, fa_p)
            _proj(nc, pools, pwkv_sb[:, p, :],
                  [xkv_c[:, j * MB:(j + 1) * MB] for j in range(2)], g_p)
            wvt_p = pwvt.tile([128, N_BANDS_P, 48], F32, tag="wvt_p")
            _wvt(nc, pools, xkv_c, pwcomb_sb[:, p, :], wvt_p, N_BANDS_P)

            acc_t = psum_acc.tile([128, MB], F32, tag=f"acc{p % 4}")
            acc_slots = [(acc_t, PACK_LO), (acc_t, PACK_HI)]
            _attn_unit(nc, tc, pools, fa_p, g_p, wvt_p, acc_slots,
                       N_BANDS_P, NP, tag="p")

            yp = ystage.tile([C, NP], F32, tag="yp")
            nc.vector.tensor_copy(yp[:, 0:MB], acc_t[PACK_LO, :])
            nc.vector.tensor_copy(yp[:, MB:NP], acc_t[PACK_HI, :])
            nc.sync.dma_start(out=ar_in_p[:, p * NP:(p + 1) * NP], in_=yp)

        # ---- AllReduce #1 (patch partials) -- overlaps the global unit
        nc.gpsimd.collective_compute(
            "AllReduce", mybir.AluOpType.add, replica_groups=REPLICA_GROUPS,
            ins=[ar_in_p.opt()], outs=[ar_out_p.opt()],
        )

        # ---- combine: residual into canvas interior
        interior = canvas[:, 1:H + 1, 1:W + 1]
        nc.vector.tensor_copy(interior, xq_hw)

        # ---- patch combine (depends on AR#1; overlaps global attention)
        for p in range(9):
            r, c0 = ROW[p], COL[p]
            yp_sb = ystage.tile([C, NP], F32, tag="yp")
            nc.sync.dma_start(out=yp_sb, in_=ar_out_p[:, p * NP:(p + 1) * NP])
            nc.scalar.activation(out=yp_sb, in_=yp_sb, func=AF.Relu,
                                 bias=shiftp_sb[:, p:p + 1], scale=1.0)
            nc.vector.tensor_scalar_mul(yp_sb, yp_sb, gvecp_sb[:, p:p + 1])
            view = canvas[:, 1 + r:1 + r + h2, 1 + c0:1 + c0 + w2]
            nc.vector.tensor_add(
                view, view, yp_sb.rearrange("c (a b) -> c a b", a=h2))

        # ================= global unit =================
        fa_sb = proj.tile([C, N], F32, tag="fa")
        g_sb = proj.tile([C, N], F32, tag="g")
        _proj(nc, pools, wq_sb,
              [xq_sb[:, j * MB:(j + 1) * MB] for j in range(N // MB)], fa_sb)
        _proj(nc, pools, wkv_sb,
              [xkv_sb[:, j * MB:(j + 1) * MB] for j in range(N // MB)], g_sb)
        wvt_g = wvt.tile([128, N_BANDS_G, 48], F32)
        _wvt(nc, pools, xkv_sb, wcomb_sb, wvt_g, N_BANDS_G)

        gacc = [psum_acc.tile([128, MB], F32, tag=f"acc{i}", name=f"gacc{i}")
                for i in range(4)]
        acc_slots = [(gacc[mb // 2], PACK_LO if mb % 2 == 0 else PACK_HI)
                     for mb in range(8)]
        _attn_unit(nc, tc, pools, fa_sb, g_sb, wvt_g, acc_slots,
                   N_BANDS_G, N, tag="g")

        yg = proj.tile([C, N], F32, tag="yg")
        for mb in range(8):
            acc_t, psl = acc_slots[mb]
            nc.vector.tensor_copy(yg[:, mb * MB:(mb + 1) * MB], acc_t[psl, :])
        nc.sync.dma_start(out=ar_in_g, in_=yg)

        # ---- AllReduce #2 (global partials)
        nc.gpsimd.collective_compute(
            "AllReduce", mybir.AluOpType.add, replica_groups=REPLICA_GROUPS,
            ins=[ar_in_g.opt()], outs=[ar_out_g.opt()],
        )

        # ---- global combine
        yg_sb = proj.tile([C, N], F32, tag="yg")
        nc.sync.dma_start(out=yg_sb, in_=ar_out_g)
        nc.scalar.activation(out=yg_sb, in_=yg_sb, func=AF.Relu,
                             bias=fuse_shift_sb, scale=1.0)
        nc.vector.tensor_scalar_mul(yg_sb, yg_sb, gvec_all_sb)
        nc.vector.tensor_add(interior, interior,
                             yg_sb.rearrange("c (h w) -> c h w", h=H))

        # ---- 3x3 conv (+ folded BN + relu)
        RB = 8  # output rows per block
        for rb in range(H // RB):
            cps = psum_acc.tile([128, RB * W], F32, tag=f"acc{rb % 4}")
            for k in range(9):
                ky, kx = k // 3, k % 3
                rhs = canvas[:, rb * RB + ky:rb * RB + ky + RB, kx:kx + W]
                nc.tensor.matmul(cps[0:C, :], convw_sb[:, k, :], rhs,
                                 start=(k == 0), stop=(k == 8),
                                 skip_group_check=True)
            ob = comb.tile([C, RB * W], F32, tag="ob")
            nc.scalar.activation(out=ob, in_=cps[0:C, :], func=AF.Relu,
                                 bias=conv_shift_sb, scale=1.0)
            nc.sync.dma_start(out=out_d[:, rb * RB:(rb + 1) * RB, :],
                              in_=ob.rearrange("c (a b) -> c a b", a=RB))

    nc.compile()
    return nc


_NC_CACHE = None


def _get_nc():
    global _NC_CACHE
    if _NC_CACHE is None:
        _NC_CACHE = build_program()
    return _NC_CACHE


def make_in_maps(inputs):
    f32 = lambda a: np.ascontiguousarray(a, np.float32)
    feats = [inputs["input_feature"], inputs["feat_b"], inputs["feat_c"],
             inputs["feat_d"]]
    W_all = [inputs["W_fa_all"], inputs["W_fb_all"], inputs["W_fc_all"],
             inputs["W_fd_all"]]
    W_p = [inputs["W_fa"], inputs["W_fb"], inputs["W_fc"], inputs["W_fd"]]

    s_fuse = inputs["fuse_g"] / np.sqrt(inputs["fuse_v"] + EPS)
    fuse_shift = (inputs["fuse_b"] - inputs["fuse_m"]) * s_fuse + inputs["fuse_bb"]
    sP = inputs["fuseP_g"] / np.sqrt(inputs["fuseP_v"] + EPS)
    shiftP = (inputs["fuseP_b"] - inputs["fuseP_m"]) * sP + inputs["fuseP_bb"]
    s_out = inputs["out_g"] / np.sqrt(inputs["out_v"] + EPS)
    conv_shift = (inputs["out_b"] - inputs["out_m"]) * s_out + inputs["out_bb"]
    conv_w = inputs["out_w"] * s_out[:, None, None, None]

    in_maps = []
    for core in range(NCORES):
        b, pair = core // 4, core % 4
        fuse_blk = s_fuse[:, None] * inputs["fuse_w"][:, pair * C:(pair + 1) * C]
        wcomb = fuse_blk @ W_all[pair]
        pwq = np.stack([W_p[0][p].T for p in range(9)])
        pwkv = np.stack([W_p[pair][p].T for p in range(9)])
        pwcomb = np.stack([
            ((sP[p][:, None] * inputs["fuseP_w"][p][:, pair * C:(pair + 1) * C])
             @ W_p[pair][p]).T
            for p in range(9)])
        convw = np.stack([conv_w[:, :, k // 3, k % 3].T for k in range(9)])
        m = dict(
            xq=f32(feats[0][b].reshape(C, N)),
            xkv=f32(feats[pair][b].reshape(C, N)),
            wq_lhsT=f32(W_all[0].T),
            wkv_lhsT=f32(W_all[pair].T),
            wcomb_rhs=f32(wcomb.T),
            pwq_lhsT=f32(pwq),
            pwkv_lhsT=f32(pwkv),
            pwcomb_rhs=f32(pwcomb),
            fuse_shift=f32(fuse_shift[:, None]),
            gvec_all=f32(np.full((C, 1), float(inputs["gamma_all"]))),
            shiftP=f32(shiftP.T),
            gvecP=f32(np.repeat((0.5 * inputs["gamma_p"])[None, :], C, 0)),
            convw_lhsT=f32(convw),
            conv_shift=f32(conv_shift[:, None]),
        )
        in_maps.append(m)
    return in_maps


def kernel(**inputs) -> np.ndarray:
    nc = _get_nc()
    in_maps = make_in_maps(inputs)
    res = run_bass_kernel_spmd(nc, in_maps, core_ids=list(range(NCORES)))
    out = np.stack([res.results[0]["out"], res.results[4]["out"]])
    return out.astype(np.float32)
